# revision 27
# baseline (speedup 1.0000x reference)
"""CategoryAttention (softmax over heads axis) on 8 Trainium2 cores.

Sharding: B*L = 4096 query rows split 8 ways (512 rows/core). Core c
handles batch b=c//4, query rows [(c%4)*512, (c%4+1)*512). Softmax is
over the 16 heads (local per (q,k) position) -> no cross-core comm.
Each core recomputes K/V projections for its batch (4x redundant).

Layout/schedule:
- All projections and attention matmuls in bf16 (FWL weight loads).
- Projections are FUSED into the attention sweep: K-chunk kn+1 and
  V-chunk kn are emitted between attention k-tiles so PE back-fills
  the ACT(exp)/DVE(normalize) pacing gaps and the HAM clock-gate
  rarely re-throttles.
- Energy matmuls row-pack two heads (partitions 0-63/64-127 ->
  concurrent PE row-groups); AV matmuls col-pack (psum halves).
- Proj psum drains use 1-bank tiles so drain overlaps accumulation.
- The reciprocal->bf16 cast runs on DVE (0.5us) instead of GPSIMD
  (1.9us): it sits on the per-k-tile critical chain.
- PE warm-keeper matmuls at kernel start (during input DMAs) and
  before the output projection defeat the HAM cold-clock penalty.
"""

import numpy as np
from contextlib import ExitStack

import concourse.bass as bass
import concourse.tile as tile
from concourse import bacc, mybir
from concourse.bass_utils import run_bass_kernel_spmd

F32 = mybir.dt.float32
BF16 = mybir.dt.bfloat16

N_CORES = 8
P = 128
D = 1024          # d_model
S = D // P        # 8 subtiles of the contraction dim
H = 16            # heads
HD = 64           # head dim
B = 2
L = 2048
LQ = L * B // N_CORES   # 512 query rows per core
LK = L                  # key rows per core (full batch slice)
KTS = 128               # k tile
NKT = LK // KTS         # 16
SCALE = 1.0 / np.sqrt(HD)

import os
BENCH_LOOP = int(os.environ.get("BENCH_LOOP", "1"))


def _build(has_bias):
    nc = bacc.Bacc("TRN2", target_bir_lowering=False, debug=False, num_devices=1)

    def din(name, shape, dt):
        return nc.dram_tensor(name, shape, dt, kind="ExternalInput").ap()

    qT_d = din("qT", (P, S * LQ), BF16)
    kT_d = din("kT", (P, 4 * S * 512), BF16)
    vT_d = din("vT", (P, 4 * S * 512), BF16)
    wq_d = din("wq", (P, 2 * S * 512), BF16)
    wk_d = din("wk", (P, 2 * S * 512), BF16)
    wv_d = din("wv", (P, 2 * S * 512), BF16)
    wo_d = din("wo", (P, 2 * S * 512), BF16)
    bias_d = {}
    for nm in ("bq", "bk", "bv", "bo"):
        if has_bias[nm]:
            bias_d[nm] = din(nm, (1, D), F32)
    outT_d = nc.dram_tensor("outT", (P, S * LQ), F32, kind="ExternalOutput").ap()

    qT_ap = qT_d.rearrange("p (s q) -> p s q", s=S)
    kT_ap = kT_d.rearrange("p (c s k) -> p c s k", c=4, s=S)
    vT_ap = vT_d.rearrange("p (c s k) -> p c s k", c=4, s=S)
    wq_ap = wq_d.rearrange("p (h s o) -> p h s o", h=2, s=S)
    wk_ap = wk_d.rearrange("p (h s o) -> p h s o", h=2, s=S)
    wv_ap = wv_d.rearrange("p (h s o) -> p h s o", h=2, s=S)
    wo_ap = wo_d.rearrange("p (h s o) -> p h s o", h=2, s=S)
    outT_ap = outT_d.rearrange("p (j q) -> p j q", j=S)

    with tile.TileContext(nc) as tc, ExitStack() as ctx:
        if BENCH_LOOP > 1:
            ctx.enter_context(tc.For_i(0, BENCH_LOOP, 1))

        # ---- persistent data tiles ----
        qt_pool = ctx.enter_context(tc.tile_pool(name="QT", bufs=1))
        kt_pool = ctx.enter_context(tc.tile_pool(name="KT", bufs=1))
        v_pool = ctx.enter_context(tc.tile_pool(name="V", bufs=1))
        QT_sb = qt_pool.tile([P, S, LQ], BF16)
        KT_sb = kt_pool.tile([P, S, LK], BF16)
        V_sb = v_pool.tile([P, NKT, D], BF16)

        any_bias = any(has_bias.values())
        bias_t = {}
        ones_t = None
        if any_bias:
            cpool = ctx.enter_context(tc.tile_pool(name="const", bufs=1))
            ones_t = cpool.tile([1, 512], F32, tag="ones")
            nc.vector.memset(ones_t[:], 1.0)
            for nm, d_ap in bias_d.items():
                t = cpool.tile([1, D], F32, tag=f"bias_{nm}")
                nc.sync.dma_start(t[:], d_ap)
                bias_t[nm] = t

        def bias_mm(ps_t, bias_name, o0, n_sz, o_on_partitions):
            if o_on_partitions:
                nc.tensor.matmul(ps_t, lhsT=bias_t[bias_name][0:1, o0:o0 + P],
                                 rhs=ones_t[0:1, :n_sz], start=False, stop=True)
            else:
                nc.tensor.matmul(ps_t, lhsT=ones_t[0:1, 0:P],
                                 rhs=bias_t[bias_name][0:1, o0:o0 + n_sz],
                                 start=False, stop=True)

        # psum pools (8 banks total: 2 proj + 4 energy + 2 av)
        ppsum = ctx.enter_context(tc.tile_pool(name="ppsum", bufs=2, space="PSUM"))
        e_psum = ctx.enter_context(tc.tile_pool(name="epsum", bufs=2, space="PSUM"))
        av_psum = ctx.enter_context(tc.tile_pool(name="avpsum", bufs=1, space="PSUM"))

        # PE warm-up: the HAM clock-gate boots at reduced rate; ~5us of dead
        # matmuls during the initial input DMAs un-throttle it so the first
        # projection matmuls run at full clock.
        warm_pool = ctx.enter_context(tc.tile_pool(name="warm", bufs=1))
        wrm = warm_pool.tile([P, 512], BF16, tag="wrm")
        nc.vector.memset(wrm[:], 0.0)
        for wk_i in range(12):
            wps = e_psum.tile([P, 2, LQ], F32, tag="e")
            for hh in range(2):
                nc.tensor.matmul(
                    wps[:, hh, :],
                    lhsT=wrm[:, 0:P],
                    rhs=wrm[:, :],
                    start=True,
                    stop=True,
                )

        # ---------------- Q projection (scoped: SBUF reused later) ----
        with tc.tile_pool(name="qstream", bufs=1) as qspool, \
             tc.tile_pool(name="qwpool", bufs=2) as qwpool:
            qin = qspool.tile([P, S, LQ], BF16, tag="qin")
            nc.sync.dma_start(qin[:], qT_ap)
            wq_h = []
            for wh in range(2):
                t = qwpool.tile([P, S, 512], BF16, tag="wq")
                nc.sync.dma_start(t[:], wq_ap[:, wh])
                wq_h.append(t)
            for j in range(S):
                ps = ppsum.tile([P, 1, 512], F32, tag="pp")
                w_t = wq_h[j // 4]
                jl = j % 4
                for s in range(S):
                    nc.tensor.matmul(
                        ps[:, 0, :LQ],
                        lhsT=w_t[:, s, jl * P:(jl + 1) * P],
                        rhs=qin[:, s, :],
                        start=(s == 0),
                        stop=(s == S - 1 and not has_bias["bq"]),
                    )
                if has_bias["bq"]:
                    bias_mm(ps[:, 0, :LQ], "bq", j * P, LQ, True)
                nc.scalar.copy(QT_sb[:, j, :], ps[:, 0, :LQ])

        # ---- attention-era pools (allocated after Q scope frees) ----
        wk_pool = ctx.enter_context(tc.tile_pool(name="wk", bufs=2))
        kin_pool = ctx.enter_context(tc.tile_pool(name="kin", bufs=1))
        wv_pool = ctx.enter_context(tc.tile_pool(name="wv", bufs=2))
        vin_pool = ctx.enter_context(tc.tile_pool(name="vin", bufs=1))
        wo_pool = ctx.enter_context(tc.tile_pool(name="wo", bufs=1))
        attn_pool = ctx.enter_context(tc.tile_pool(name="attn", bufs=3))
        tree_pool = ctx.enter_context(tc.tile_pool(name="tree", bufs=1))
        den_pool = ctx.enter_context(tc.tile_pool(name="den", bufs=1))
        r_pool = ctx.enter_context(tc.tile_pool(name="r", bufs=2))
        rb_pool = ctx.enter_context(tc.tile_pool(name="rb", bufs=2))
        ctx_pool = ctx.enter_context(tc.tile_pool(name="ctx", bufs=1))
        osb_pool = ctx.enter_context(tc.tile_pool(name="osb", bufs=2))

        ctx_sb = ctx_pool.tile([P, S, LQ], BF16)

        wk_h = []
        for wh in range(2):
            t = wk_pool.tile([P, S, 512], BF16, tag="wk")
            nc.sync.dma_start(t[:], wk_ap[:, wh])
            wk_h.append(t)
        wv_h = []
        for wh in range(2):
            t = wv_pool.tile([P, S, 512], BF16, tag="wv")
            nc.sync.dma_start(t[:], wv_ap[:, wh])
            wv_h.append(t)

        kin_cur = [None]
        vin_cur = [None]

        def dma_kin(kn):
            t = kin_pool.tile([P, S, 512], BF16, tag="kin")
            nc.sync.dma_start(t[:], kT_ap[:, kn])
            kin_cur[0] = t

        def dma_vin(kn):
            t = vin_pool.tile([P, S, 512], BF16, tag="vin")
            nc.sync.dma_start(t[:], vT_ap[:, kn])
            vin_cur[0] = t

        def k_chunk_quarter(kn, jq):
            """Project K columns [kn*512,(kn+1)*512) for o-tiles 2jq,2jq+1."""
            kin = kin_cur[0]
            for j in (2 * jq, 2 * jq + 1):
                ps = ppsum.tile([P, 1, 512], F32, tag="pp")
                w_t = wk_h[j // 4]
                jl = j % 4
                for s in range(S):
                    nc.tensor.matmul(
                        ps[:, 0, :],
                        lhsT=w_t[:, s, jl * P:(jl + 1) * P],
                        rhs=kin[:, s, :],
                        start=(s == 0),
                        stop=(s == S - 1 and not has_bias["bk"]),
                    )
                if has_bias["bk"]:
                    bias_mm(ps[:, 0, :], "bk", j * P, 512, True)
                nc.scalar.copy(KT_sb[:, j, kn * 512:(kn + 1) * 512], ps[:, 0, :])

        def v_chunk_quarter(kn, kt4):
            """Project V rows for k-tile kn*4+kt4 (128 rows, all 1024 cols)."""
            vin = vin_cur[0]
            kt = kn * 4 + kt4
            for t in range(2):
                ps = ppsum.tile([P, 1, 512], F32, tag="pp")
                for s in range(S):
                    nc.tensor.matmul(
                        ps[:, 0, :],
                        lhsT=vin[:, s, kt4 * P:(kt4 + 1) * P],
                        rhs=wv_h[t][:, s, :],
                        start=(s == 0),
                        stop=(s == S - 1 and not has_bias["bv"]),
                    )
                if has_bias["bv"]:
                    bias_mm(ps[:, 0, :], "bv", t * 512, 512, False)
                nc.scalar.copy(V_sb[:, kt, t * 512:(t + 1) * 512], ps[:, 0, :])

        # ---------------- attention ----------------
        def softmax_kt(kt):
            """Energy (16 heads, row-packed pairs) -> exp -> normalized attn."""
            attn_t = attn_pool.tile([P, H, LQ], BF16, tag="attn")
            for g in range(8):
                eps = e_psum.tile([P, 2, LQ], F32, tag="e")
                for hh in range(2):
                    p0 = HD * hh
                    nc.tensor.matmul(
                        eps[:, hh, :],
                        lhsT=KT_sb[p0:p0 + HD, g, kt * KTS:(kt + 1) * KTS],
                        rhs=QT_sb[p0:p0 + HD, g, :],
                        start=True,
                        stop=True,
                    )
                nc.scalar.activation(attn_t[:, g * 2:(g + 1) * 2, :], eps[:],
                                     mybir.ActivationFunctionType.Exp,
                                     scale=float(SCALE))
            # den = sum over heads (bf16 tree at DVE 2x; final add f32)
            t1 = tree_pool.tile([P, 4, LQ], BF16)
            with nc.allow_low_precision(reason="bf16 head-sum tree"):
                nc.vector.tensor_add(t1[:], attn_t[:, 0:4, :], attn_t[:, 4:8, :])
                nc.vector.tensor_add(t1[:], t1[:], attn_t[:, 8:12, :])
                nc.vector.tensor_add(t1[:], t1[:], attn_t[:, 12:16, :])
                nc.vector.tensor_add(t1[:, 0:2, :], t1[:, 0:2, :], t1[:, 2:4, :])
            den = den_pool.tile([P, LQ], F32)
            nc.vector.tensor_add(den[:], t1[:, 0, :], t1[:, 1, :])
            r32 = r_pool.tile([P, LQ], F32, tag="r")
            nc.vector.reciprocal_approx_fast(r32[:], den[:])
            rb = rb_pool.tile([P, LQ], BF16, tag="rb")
            with nc.allow_low_precision(reason="bf16 reciprocal"):
                nc.vector.tensor_copy(rb[:], r32[:])
            nc.vector.tensor_mul(
                attn_t[:], attn_t[:],
                rb[:, None, :].to_broadcast((P, H, LQ)))
            return attn_t

        def av_group(u, c0, attn_list, first):
            """One avp tile: heads 4u..4u+3, full q, over 2 k-tiles."""
            avp = av_psum.tile([P, 2, LQ], F32, tag="av")
            for ci in range(2):
                kt = c0 + ci
                for hh in range(4):
                    h = 4 * u + hh
                    i, p0 = hh // 2, HD * (hh % 2)
                    nc.tensor.matmul(
                        avp[p0:p0 + HD, i, :],
                        lhsT=V_sb[:, kt, h * HD:(h + 1) * HD],
                        rhs=attn_list[ci][:, h, :],
                        start=(ci == 0),
                        stop=(ci == 1),
                    )
            with nc.allow_low_precision(reason="bf16 ctx accumulate"):
                if first:
                    nc.vector.tensor_copy(ctx_sb[:, 2 * u:2 * u + 2, :],
                                          avp[:, :, :])
                else:
                    nc.vector.tensor_add(ctx_sb[:, 2 * u:2 * u + 2, :],
                                         ctx_sb[:, 2 * u:2 * u + 2, :],
                                         avp[:, :, :])

        wo_tiles = []

        def dma_wo0():
            t = wo_pool.tile([P, S, 512], BF16, tag="wo")
            nc.sync.dma_start(t[:], wo_ap[:, 0])
            wo_tiles.append(t)

        # filler schedule: per-kt projection quarters + input DMAs
        def proj_filler(kt):
            if kt == 0:
                dma_kin(1)
                v_chunk_quarter(0, 0); v_chunk_quarter(0, 1)
            elif kt == 1:
                v_chunk_quarter(0, 2); v_chunk_quarter(0, 3)
                dma_vin(1)
            elif kt in (2, 3, 6, 7, 10, 11):
                kn = kt // 4 + 1
                jq0 = 0 if kt % 4 == 2 else 2
                k_chunk_quarter(kn, jq0); k_chunk_quarter(kn, jq0 + 1)
                if kt in (3, 7):
                    dma_kin(kn + 1)
            elif kt in (4, 5, 8, 9, 12, 13):
                kn = kt // 4
                kt40 = 0 if kt % 4 == 0 else 2
                v_chunk_quarter(kn, kt40); v_chunk_quarter(kn, kt40 + 1)
                if kt in (5, 9):
                    dma_vin(kn + 1)
                if kt == 12:
                    dma_wo0()
            # kt 14, 15: no proj work left

        # prologue: K chunk 0 (all 8 o-tiles)
        dma_kin(0)
        dma_vin(0)
        k_chunk_quarter(0, 0); k_chunk_quarter(0, 1)
        k_chunk_quarter(0, 2); k_chunk_quarter(0, 3)

        prev = None  # (c0, [attn_kt0, attn_kt1])
        for p in range(8):
            c0 = 2 * p
            cur = []
            for ci in range(2):
                kt = c0 + ci
                cur.append(softmax_kt(kt))
                if prev is not None:
                    for u in (2 * ci, 2 * ci + 1):
                        av_group(u, prev[0], prev[1], prev[0] == 0)
                proj_filler(kt)
            prev = (c0, cur)
        # PE warm-keeper: ~4us of dead matmuls run while the DVE finishes
        # the last normalize, so the HAM clock-gate stays at full rate for
        # the final AV groups + output projection.
        for wk_i in range(10):
            wps = e_psum.tile([P, 2, LQ], F32, tag="e")
            for hh in range(2):
                p0 = HD * hh
                nc.tensor.matmul(
                    wps[:, hh, :],
                    lhsT=KT_sb[p0:p0 + HD, wk_i % S, 0:KTS],
                    rhs=QT_sb[p0:p0 + HD, wk_i % S, :],
                    start=True,
                    stop=True,
                )
        for u in range(4):
            av_group(u, prev[0], prev[1], False)

        # ---------------- output projection ----------------
        for j4 in range(2):
            if j4 == 0 and wo_tiles:
                woh = wo_tiles[0]
            else:
                woh = wo_pool.tile([P, S, 512], BF16, tag="wo")
                nc.sync.dma_start(woh[:], wo_ap[:, j4])
            for j2 in range(2):
                po = e_psum.tile([P, 2, LQ], F32, tag="e")
                for jj in range(2):
                    j = j4 * 4 + j2 * 2 + jj
                    jl = j2 * 2 + jj
                    for s in range(S):
                        nc.tensor.matmul(
                            po[:, jj, :],
                            lhsT=woh[:, s, jl * P:(jl + 1) * P],
                            rhs=ctx_sb[:, s, :],
                            start=(s == 0),
                            stop=(s == S - 1 and not has_bias["bo"]),
                        )
                    if has_bias["bo"]:
                        bias_mm(po[:, jj, :], "bo", j * P, LQ, True)
                osb = osb_pool.tile([P, 2, LQ], F32, tag="osb")
                nc.scalar.copy(osb[:], po[:])
                j0 = j4 * 4 + j2 * 2
                nc.sync.dma_start(outT_ap[:, j0:j0 + 2, :], osb[:])

    nc.compile()
    return nc


_cache = {}


def _get_program(has_bias):
    key = (BENCH_LOOP, tuple(sorted(has_bias.items())))
    if key not in _cache:
        _cache[key] = _build(has_bias)
    return _cache[key]


def _part_major(x):
    n = x.shape[1]
    return np.ascontiguousarray(
        x.reshape(S, P, n).transpose(1, 0, 2).reshape(P, S * n))


def _chunked(x, width=512):
    """[D, N] -> [P, N//width, S, width] per-chunk contiguous layout."""
    n = x.shape[1]
    nch = n // width
    y = x.reshape(S, P, nch, width).transpose(1, 2, 0, 3)
    return np.ascontiguousarray(y.reshape(P, nch * S * width))


def _bf16(x):
    import ml_dtypes
    return np.ascontiguousarray(x).astype(ml_dtypes.bfloat16)


def prepare_inputs(query, key, value, Wq_w, Wq_b, Wk_w, Wk_b, Wv_w, Wv_b,
                   Wo_w, Wo_b):
    query = np.asarray(query, dtype=np.float32)
    key = np.asarray(key, dtype=np.float32)
    value = np.asarray(value, dtype=np.float32)
    w = {
        "wq": _bf16(_chunked(np.ascontiguousarray(np.asarray(Wq_w, np.float32).T))),
        "wk": _bf16(_chunked(np.ascontiguousarray(np.asarray(Wk_w, np.float32).T))),
        "wv": _bf16(_chunked(np.ascontiguousarray(np.asarray(Wv_w, np.float32).T))),
        "wo": _bf16(_chunked(np.ascontiguousarray(np.asarray(Wo_w, np.float32).T))),
    }
    biases = {"bq": np.asarray(Wq_b, np.float32), "bk": np.asarray(Wk_b, np.float32),
              "bv": np.asarray(Wv_b, np.float32), "bo": np.asarray(Wo_b, np.float32)}
    has_bias = {nm: bool(np.any(b)) for nm, b in biases.items()}

    kT = [_bf16(_chunked(np.ascontiguousarray(key[b].T))) for b in range(B)]
    vT = [_bf16(_chunked(np.ascontiguousarray(value[b].T))) for b in range(B)]

    in_maps = []
    for c in range(N_CORES):
        b, qc = c // (N_CORES // B), c % (N_CORES // B)
        qslice = query[b, qc * LQ:(qc + 1) * LQ, :]
        m = {
            "qT": _bf16(_part_major(np.ascontiguousarray(qslice.T))),
            "kT": kT[b],
            "vT": vT[b],
            **w,
        }
        for nm, hb in has_bias.items():
            if hb:
                m[nm] = biases[nm].reshape(1, D)
        in_maps.append(m)
    return in_maps, has_bias


def gather_output(results):
    out = np.empty((B, L, D), dtype=np.float32)
    for c in range(N_CORES):
        b, qc = c // (N_CORES // B), c % (N_CORES // B)
        oT = results[c]["outT"].reshape(P, S, LQ).transpose(1, 0, 2).reshape(D, LQ)
        out[b, qc * LQ:(qc + 1) * LQ, :] = oT.T
    return out


def kernel(**inputs) -> np.ndarray:
    in_maps, has_bias = prepare_inputs(**inputs)
    nc = _get_program(has_bias)
    res = run_bass_kernel_spmd(nc, in_maps, list(range(N_CORES)))
    return gather_output(res.results)


# revision 29
# speedup vs baseline: 1.0018x; 1.0018x over previous
"""CategoryAttention (softmax over heads axis) on 8 Trainium2 cores.

Sharding: B*L = 4096 query rows split 8 ways (512 rows/core). Core c
handles batch b=c//4, query rows [(c%4)*512, (c%4+1)*512). Softmax is
over the 16 heads (local per (q,k) position) -> no cross-core comm.
Each core recomputes K/V projections for its batch (4x redundant).

Layout/schedule:
- All projections and attention matmuls in bf16 (FWL weight loads).
- Projections are FUSED into the attention sweep: K-chunk kn+1 and
  V-chunk kn are emitted between attention k-tiles so PE back-fills
  the ACT(exp)/DVE(normalize) pacing gaps and the HAM clock-gate
  rarely re-throttles.
- Energy matmuls row-pack two heads (partitions 0-63/64-127 ->
  concurrent PE row-groups); AV matmuls col-pack (psum halves).
- Proj psum drains use 1-bank tiles so drain overlaps accumulation.
- The reciprocal->bf16 cast runs on DVE (0.5us) instead of GPSIMD
  (1.9us): it sits on the per-k-tile critical chain.
- PE warm-keeper matmuls at kernel start (during input DMAs) and
  before the output projection defeat the HAM cold-clock penalty.
"""

import numpy as np
from contextlib import ExitStack

import concourse.bass as bass
import concourse.tile as tile
from concourse import bacc, mybir
from concourse.bass_utils import run_bass_kernel_spmd

F32 = mybir.dt.float32
BF16 = mybir.dt.bfloat16

N_CORES = 8
P = 128
D = 1024          # d_model
S = D // P        # 8 subtiles of the contraction dim
H = 16            # heads
HD = 64           # head dim
B = 2
L = 2048
LQ = L * B // N_CORES   # 512 query rows per core
LK = L                  # key rows per core (full batch slice)
KTS = 128               # k tile
NKT = LK // KTS         # 16
SCALE = 1.0 / np.sqrt(HD)

import os
BENCH_LOOP = int(os.environ.get("BENCH_LOOP", "1"))


def _build(has_bias):
    nc = bacc.Bacc("TRN2", target_bir_lowering=False, debug=False, num_devices=1)

    def din(name, shape, dt):
        return nc.dram_tensor(name, shape, dt, kind="ExternalInput").ap()

    qT_d = din("qT", (P, S * LQ), BF16)
    kT_d = din("kT", (P, 4 * S * 512), BF16)
    vT_d = din("vT", (P, 4 * S * 512), BF16)
    wq_d = din("wq", (P, 2 * S * 512), BF16)
    wk_d = din("wk", (P, 2 * S * 512), BF16)
    wv_d = din("wv", (P, 2 * S * 512), BF16)
    wo_d = din("wo", (P, 2 * S * 512), BF16)
    bias_d = {}
    for nm in ("bq", "bk", "bv", "bo"):
        if has_bias[nm]:
            bias_d[nm] = din(nm, (1, D), F32)
    outT_d = nc.dram_tensor("outT", (P, S * LQ), F32, kind="ExternalOutput").ap()

    qT_ap = qT_d.rearrange("p (s q) -> p s q", s=S)
    kT_ap = kT_d.rearrange("p (c s k) -> p c s k", c=4, s=S)
    vT_ap = vT_d.rearrange("p (c s k) -> p c s k", c=4, s=S)
    wq_ap = wq_d.rearrange("p (h s o) -> p h s o", h=2, s=S)
    wk_ap = wk_d.rearrange("p (h s o) -> p h s o", h=2, s=S)
    wv_ap = wv_d.rearrange("p (h s o) -> p h s o", h=2, s=S)
    wo_ap = wo_d.rearrange("p (h s o) -> p h s o", h=2, s=S)
    outT_ap = outT_d.rearrange("p (j q) -> p j q", j=S)

    with tile.TileContext(nc) as tc, ExitStack() as ctx:
        if BENCH_LOOP > 1:
            ctx.enter_context(tc.For_i(0, BENCH_LOOP, 1))

        # ---- persistent data tiles ----
        qt_pool = ctx.enter_context(tc.tile_pool(name="QT", bufs=1))
        kt_pool = ctx.enter_context(tc.tile_pool(name="KT", bufs=1))
        v_pool = ctx.enter_context(tc.tile_pool(name="V", bufs=1))
        QT_sb = qt_pool.tile([P, S, LQ], BF16)
        KT_sb = kt_pool.tile([P, S, LK], BF16)
        V_sb = v_pool.tile([P, NKT, D], BF16)

        any_bias = any(has_bias.values())
        bias_t = {}
        ones_t = None
        if any_bias:
            cpool = ctx.enter_context(tc.tile_pool(name="const", bufs=1))
            ones_t = cpool.tile([1, 512], F32, tag="ones")
            nc.vector.memset(ones_t[:], 1.0)
            for nm, d_ap in bias_d.items():
                t = cpool.tile([1, D], F32, tag=f"bias_{nm}")
                nc.sync.dma_start(t[:], d_ap)
                bias_t[nm] = t

        def bias_mm(ps_t, bias_name, o0, n_sz, o_on_partitions):
            if o_on_partitions:
                nc.tensor.matmul(ps_t, lhsT=bias_t[bias_name][0:1, o0:o0 + P],
                                 rhs=ones_t[0:1, :n_sz], start=False, stop=True)
            else:
                nc.tensor.matmul(ps_t, lhsT=ones_t[0:1, 0:P],
                                 rhs=bias_t[bias_name][0:1, o0:o0 + n_sz],
                                 start=False, stop=True)

        # psum pools (8 banks total: 2 proj + 4 energy + 2 av)
        ppsum = ctx.enter_context(tc.tile_pool(name="ppsum", bufs=2, space="PSUM"))
        e_psum = ctx.enter_context(tc.tile_pool(name="epsum", bufs=2, space="PSUM"))
        av_psum = ctx.enter_context(tc.tile_pool(name="avpsum", bufs=1, space="PSUM"))

        # PE warm-up: the HAM clock-gate boots at reduced rate; ~5us of dead
        # matmuls during the initial input DMAs un-throttle it so the first
        # projection matmuls run at full clock.
        warm_pool = ctx.enter_context(tc.tile_pool(name="warm", bufs=1))
        wrm = warm_pool.tile([P, 512], BF16, tag="wrm")
        nc.vector.memset(wrm[:], 0.0)
        for wk_i in range(12):
            wps = e_psum.tile([P, 2, LQ], F32, tag="e")
            for hh in range(2):
                nc.tensor.matmul(
                    wps[:, hh, :],
                    lhsT=wrm[:, 0:P],
                    rhs=wrm[:, :],
                    start=True,
                    stop=True,
                )

        # ---------------- Q projection (scoped: SBUF reused later) ----
        with tc.tile_pool(name="qstream", bufs=1) as qspool, \
             tc.tile_pool(name="qwpool", bufs=2) as qwpool:
            qin = qspool.tile([P, S, LQ], BF16, tag="qin")
            nc.sync.dma_start(qin[:], qT_ap)
            wq_h = []
            for wh in range(2):
                t = qwpool.tile([P, S, 512], BF16, tag="wq")
                nc.sync.dma_start(t[:], wq_ap[:, wh])
                wq_h.append(t)
            for j in range(S):
                ps = ppsum.tile([P, 1, 512], F32, tag="pp")
                w_t = wq_h[j // 4]
                jl = j % 4
                for s in range(S):
                    nc.tensor.matmul(
                        ps[:, 0, :LQ],
                        lhsT=w_t[:, s, jl * P:(jl + 1) * P],
                        rhs=qin[:, s, :],
                        start=(s == 0),
                        stop=(s == S - 1 and not has_bias["bq"]),
                    )
                if has_bias["bq"]:
                    bias_mm(ps[:, 0, :LQ], "bq", j * P, LQ, True)
                nc.scalar.copy(QT_sb[:, j, :], ps[:, 0, :LQ])

        # ---- attention-era pools (allocated after Q scope frees) ----
        wk_pool = ctx.enter_context(tc.tile_pool(name="wk", bufs=2))
        kin_pool = ctx.enter_context(tc.tile_pool(name="kin", bufs=1))
        wv_pool = ctx.enter_context(tc.tile_pool(name="wv", bufs=2))
        vin_pool = ctx.enter_context(tc.tile_pool(name="vin", bufs=1))
        wo_pool = ctx.enter_context(tc.tile_pool(name="wo", bufs=1))
        attn_pool = ctx.enter_context(tc.tile_pool(name="attn", bufs=3))
        tree_pool = ctx.enter_context(tc.tile_pool(name="tree", bufs=1))
        den_pool = ctx.enter_context(tc.tile_pool(name="den", bufs=1))
        r_pool = ctx.enter_context(tc.tile_pool(name="r", bufs=2))
        rb_pool = ctx.enter_context(tc.tile_pool(name="rb", bufs=2))
        ctx_pool = ctx.enter_context(tc.tile_pool(name="ctx", bufs=1))
        osb_pool = ctx.enter_context(tc.tile_pool(name="osb", bufs=2))

        ctx_sb = ctx_pool.tile([P, S, LQ], BF16)

        wk_h = []
        for wh in range(2):
            t = wk_pool.tile([P, S, 512], BF16, tag="wk")
            nc.sync.dma_start(t[:], wk_ap[:, wh])
            wk_h.append(t)
        wv_h = []
        for wh in range(2):
            t = wv_pool.tile([P, S, 512], BF16, tag="wv")
            nc.sync.dma_start(t[:], wv_ap[:, wh])
            wv_h.append(t)

        kin_cur = [None]
        vin_cur = [None]

        def dma_kin(kn):
            t = kin_pool.tile([P, S, 512], BF16, tag="kin")
            nc.sync.dma_start(t[:], kT_ap[:, kn])
            kin_cur[0] = t

        def dma_vin(kn):
            t = vin_pool.tile([P, S, 512], BF16, tag="vin")
            nc.sync.dma_start(t[:], vT_ap[:, kn])
            vin_cur[0] = t

        def k_chunk_quarter(kn, jq):
            """Project K columns [kn*512,(kn+1)*512) for o-tiles 2jq,2jq+1."""
            kin = kin_cur[0]
            for j in (2 * jq, 2 * jq + 1):
                ps = ppsum.tile([P, 1, 512], F32, tag="pp")
                w_t = wk_h[j // 4]
                jl = j % 4
                for s in range(S):
                    nc.tensor.matmul(
                        ps[:, 0, :],
                        lhsT=w_t[:, s, jl * P:(jl + 1) * P],
                        rhs=kin[:, s, :],
                        start=(s == 0),
                        stop=(s == S - 1 and not has_bias["bk"]),
                    )
                if has_bias["bk"]:
                    bias_mm(ps[:, 0, :], "bk", j * P, 512, True)
                nc.scalar.copy(KT_sb[:, j, kn * 512:(kn + 1) * 512], ps[:, 0, :])

        def v_chunk_quarter(kn, kt4):
            """Project V rows for k-tile kn*4+kt4 (128 rows, all 1024 cols)."""
            vin = vin_cur[0]
            kt = kn * 4 + kt4
            for t in range(2):
                ps = ppsum.tile([P, 1, 512], F32, tag="pp")
                for s in range(S):
                    nc.tensor.matmul(
                        ps[:, 0, :],
                        lhsT=vin[:, s, kt4 * P:(kt4 + 1) * P],
                        rhs=wv_h[t][:, s, :],
                        start=(s == 0),
                        stop=(s == S - 1 and not has_bias["bv"]),
                    )
                if has_bias["bv"]:
                    bias_mm(ps[:, 0, :], "bv", t * 512, 512, False)
                nc.scalar.copy(V_sb[:, kt, t * 512:(t + 1) * 512], ps[:, 0, :])

        # ---------------- attention ----------------
        def softmax_kt(kt, inner=()):
            """Energy (16 heads, row-packed pairs) -> exp -> normalized attn.
            ``inner`` closures (projection chunks) are emitted after the
            matching psum group's exp: group g only reads K^T plane g, so
            K-chunk quarters can land just-in-time between groups."""
            attn_t = attn_pool.tile([P, H, LQ], BF16, tag="attn")
            inner = list(inner)
            for g in range(8):
                eps = e_psum.tile([P, 2, LQ], F32, tag="e")
                for hh in range(2):
                    p0 = HD * hh
                    nc.tensor.matmul(
                        eps[:, hh, :],
                        lhsT=KT_sb[p0:p0 + HD, g, kt * KTS:(kt + 1) * KTS],
                        rhs=QT_sb[p0:p0 + HD, g, :],
                        start=True,
                        stop=True,
                    )
                nc.scalar.activation(attn_t[:, g * 2:(g + 1) * 2, :], eps[:],
                                     mybir.ActivationFunctionType.Exp,
                                     scale=float(SCALE))
                if g < len(inner):
                    inner[g]()
            # den = sum over heads (bf16 tree at DVE 2x; final add f32)
            t1 = tree_pool.tile([P, 4, LQ], BF16)
            with nc.allow_low_precision(reason="bf16 head-sum tree"):
                nc.vector.tensor_add(t1[:], attn_t[:, 0:4, :], attn_t[:, 4:8, :])
                nc.vector.tensor_add(t1[:], t1[:], attn_t[:, 8:12, :])
                nc.vector.tensor_add(t1[:], t1[:], attn_t[:, 12:16, :])
                nc.vector.tensor_add(t1[:, 0:2, :], t1[:, 0:2, :], t1[:, 2:4, :])
            den = den_pool.tile([P, LQ], F32)
            nc.vector.tensor_add(den[:], t1[:, 0, :], t1[:, 1, :])
            r32 = r_pool.tile([P, LQ], F32, tag="r")
            nc.vector.reciprocal_approx_fast(r32[:], den[:])
            rb = rb_pool.tile([P, LQ], BF16, tag="rb")
            with nc.allow_low_precision(reason="bf16 reciprocal"):
                nc.vector.tensor_copy(rb[:], r32[:])
            nc.vector.tensor_mul(
                attn_t[:], attn_t[:],
                rb[:, None, :].to_broadcast((P, H, LQ)))
            return attn_t

        def av_group(u, c0, attn_list, first):
            """One avp tile: heads 4u..4u+3, full q, over 2 k-tiles."""
            avp = av_psum.tile([P, 2, LQ], F32, tag="av")
            for ci in range(2):
                kt = c0 + ci
                for hh in range(4):
                    h = 4 * u + hh
                    i, p0 = hh // 2, HD * (hh % 2)
                    nc.tensor.matmul(
                        avp[p0:p0 + HD, i, :],
                        lhsT=V_sb[:, kt, h * HD:(h + 1) * HD],
                        rhs=attn_list[ci][:, h, :],
                        start=(ci == 0),
                        stop=(ci == 1),
                    )
            with nc.allow_low_precision(reason="bf16 ctx accumulate"):
                if first:
                    nc.vector.tensor_copy(ctx_sb[:, 2 * u:2 * u + 2, :],
                                          avp[:, :, :])
                else:
                    nc.vector.tensor_add(ctx_sb[:, 2 * u:2 * u + 2, :],
                                         ctx_sb[:, 2 * u:2 * u + 2, :],
                                         avp[:, :, :])

        wo_tiles = []

        def dma_wo0():
            t = wo_pool.tile([P, S, 512], BF16, tag="wo")
            nc.sync.dma_start(t[:], wo_ap[:, 0])
            wo_tiles.append(t)

        # filler schedule: per-kt projection quarters + input DMAs
        def proj_filler(kt):
            if kt == 0:
                dma_kin(1)
                v_chunk_quarter(0, 0); v_chunk_quarter(0, 1)
            elif kt == 1:
                v_chunk_quarter(0, 2); v_chunk_quarter(0, 3)
                dma_vin(1)
            elif kt in (2, 3, 6, 7, 10, 11):
                kn = kt // 4 + 1
                jq0 = 0 if kt % 4 == 2 else 2
                k_chunk_quarter(kn, jq0); k_chunk_quarter(kn, jq0 + 1)
                if kt in (3, 7):
                    dma_kin(kn + 1)
            elif kt in (4, 5, 8, 9, 12, 13):
                kn = kt // 4
                kt40 = 0 if kt % 4 == 0 else 2
                v_chunk_quarter(kn, kt40); v_chunk_quarter(kn, kt40 + 1)
                if kt in (5, 9):
                    dma_vin(kn + 1)
                if kt == 12:
                    dma_wo0()
            # kt 14, 15: no proj work left

        # prologue: only K chunk 0's first quarter up-front; the other
        # three quarters land just-in-time inside softmax_kt(0) so the
        # first exps (and the DVE behind them) start ~10us earlier.
        dma_kin(0)
        dma_vin(0)
        k_chunk_quarter(0, 0)
        a0 = softmax_kt(0, inner=[lambda: k_chunk_quarter(0, 1),
                                  lambda: k_chunk_quarter(0, 2),
                                  lambda: k_chunk_quarter(0, 3)])
        a1 = softmax_kt(1)
        proj_filler(0)
        proj_filler(1)
        prev = (0, [a0, a1])

        for p in range(1, 8):
            c0 = 2 * p
            cur = []
            for ci in range(2):
                kt = c0 + ci
                cur.append(softmax_kt(kt))
                if prev is not None:
                    for u in (2 * ci, 2 * ci + 1):
                        av_group(u, prev[0], prev[1], prev[0] == 0)
                proj_filler(kt)
            prev = (c0, cur)
        # PE warm-keeper: ~4us of dead matmuls run while the DVE finishes
        # the last normalize, so the HAM clock-gate stays at full rate for
        # the final AV groups + output projection.
        for wk_i in range(10):
            wps = e_psum.tile([P, 2, LQ], F32, tag="e")
            for hh in range(2):
                p0 = HD * hh
                nc.tensor.matmul(
                    wps[:, hh, :],
                    lhsT=KT_sb[p0:p0 + HD, wk_i % S, 0:KTS],
                    rhs=QT_sb[p0:p0 + HD, wk_i % S, :],
                    start=True,
                    stop=True,
                )
        for u in range(4):
            av_group(u, prev[0], prev[1], False)

        # ---------------- output projection ----------------
        for j4 in range(2):
            if j4 == 0 and wo_tiles:
                woh = wo_tiles[0]
            else:
                woh = wo_pool.tile([P, S, 512], BF16, tag="wo")
                nc.sync.dma_start(woh[:], wo_ap[:, j4])
            for j2 in range(2):
                po = e_psum.tile([P, 2, LQ], F32, tag="e")
                for jj in range(2):
                    j = j4 * 4 + j2 * 2 + jj
                    jl = j2 * 2 + jj
                    for s in range(S):
                        nc.tensor.matmul(
                            po[:, jj, :],
                            lhsT=woh[:, s, jl * P:(jl + 1) * P],
                            rhs=ctx_sb[:, s, :],
                            start=(s == 0),
                            stop=(s == S - 1 and not has_bias["bo"]),
                        )
                    if has_bias["bo"]:
                        bias_mm(po[:, jj, :], "bo", j * P, LQ, True)
                osb = osb_pool.tile([P, 2, LQ], F32, tag="osb")
                nc.scalar.copy(osb[:], po[:])
                j0 = j4 * 4 + j2 * 2
                nc.sync.dma_start(outT_ap[:, j0:j0 + 2, :], osb[:])

    nc.compile()
    return nc


_cache = {}


def _get_program(has_bias):
    key = (BENCH_LOOP, tuple(sorted(has_bias.items())))
    if key not in _cache:
        _cache[key] = _build(has_bias)
    return _cache[key]


def _part_major(x):
    n = x.shape[1]
    return np.ascontiguousarray(
        x.reshape(S, P, n).transpose(1, 0, 2).reshape(P, S * n))


def _chunked(x, width=512):
    """[D, N] -> [P, N//width, S, width] per-chunk contiguous layout."""
    n = x.shape[1]
    nch = n // width
    y = x.reshape(S, P, nch, width).transpose(1, 2, 0, 3)
    return np.ascontiguousarray(y.reshape(P, nch * S * width))


def _bf16(x):
    import ml_dtypes
    return np.ascontiguousarray(x).astype(ml_dtypes.bfloat16)


def prepare_inputs(query, key, value, Wq_w, Wq_b, Wk_w, Wk_b, Wv_w, Wv_b,
                   Wo_w, Wo_b):
    query = np.asarray(query, dtype=np.float32)
    key = np.asarray(key, dtype=np.float32)
    value = np.asarray(value, dtype=np.float32)
    w = {
        "wq": _bf16(_chunked(np.ascontiguousarray(np.asarray(Wq_w, np.float32).T))),
        "wk": _bf16(_chunked(np.ascontiguousarray(np.asarray(Wk_w, np.float32).T))),
        "wv": _bf16(_chunked(np.ascontiguousarray(np.asarray(Wv_w, np.float32).T))),
        "wo": _bf16(_chunked(np.ascontiguousarray(np.asarray(Wo_w, np.float32).T))),
    }
    biases = {"bq": np.asarray(Wq_b, np.float32), "bk": np.asarray(Wk_b, np.float32),
              "bv": np.asarray(Wv_b, np.float32), "bo": np.asarray(Wo_b, np.float32)}
    has_bias = {nm: bool(np.any(b)) for nm, b in biases.items()}

    kT = [_bf16(_chunked(np.ascontiguousarray(key[b].T))) for b in range(B)]
    vT = [_bf16(_chunked(np.ascontiguousarray(value[b].T))) for b in range(B)]

    in_maps = []
    for c in range(N_CORES):
        b, qc = c // (N_CORES // B), c % (N_CORES // B)
        qslice = query[b, qc * LQ:(qc + 1) * LQ, :]
        m = {
            "qT": _bf16(_part_major(np.ascontiguousarray(qslice.T))),
            "kT": kT[b],
            "vT": vT[b],
            **w,
        }
        for nm, hb in has_bias.items():
            if hb:
                m[nm] = biases[nm].reshape(1, D)
        in_maps.append(m)
    return in_maps, has_bias


def gather_output(results):
    out = np.empty((B, L, D), dtype=np.float32)
    for c in range(N_CORES):
        b, qc = c // (N_CORES // B), c % (N_CORES // B)
        oT = results[c]["outT"].reshape(P, S, LQ).transpose(1, 0, 2).reshape(D, LQ)
        out[b, qc * LQ:(qc + 1) * LQ, :] = oT.T
    return out


def kernel(**inputs) -> np.ndarray:
    in_maps, has_bias = prepare_inputs(**inputs)
    nc = _get_program(has_bias)
    res = run_bass_kernel_spmd(nc, in_maps, list(range(N_CORES)))
    return gather_output(res.results)


# revision 32
# speedup vs baseline: 1.0028x; 1.0010x over previous
"""CategoryAttention (softmax over heads axis) on 8 Trainium2 cores.

Sharding: B*L = 4096 query rows split 8 ways (512 rows/core). Core c
handles batch b=c//4, query rows [(c%4)*512, (c%4+1)*512). Softmax is
over the 16 heads (local per (q,k) position) -> no cross-core comm.
Each core recomputes K/V projections for its batch (4x redundant).

Layout/schedule:
- All projections and attention matmuls in bf16 (FWL weight loads).
- Projections are FUSED into the attention sweep: K-chunk kn+1 and
  V-chunk kn are emitted between attention k-tiles so PE back-fills
  the ACT(exp)/DVE(normalize) pacing gaps and the HAM clock-gate
  rarely re-throttles.
- Energy matmuls row-pack two heads (partitions 0-63/64-127 ->
  concurrent PE row-groups); AV matmuls col-pack (psum halves).
- Proj psum drains use 1-bank tiles so drain overlaps accumulation.
- The reciprocal->bf16 cast runs on DVE (0.5us) instead of GPSIMD
  (1.9us): it sits on the per-k-tile critical chain.
- PE warm-keeper matmuls at kernel start (during input DMAs) and
  before the output projection defeat the HAM cold-clock penalty.
"""

import numpy as np
from contextlib import ExitStack

import concourse.bass as bass
import concourse.tile as tile
from concourse import bacc, mybir
from concourse.bass_utils import run_bass_kernel_spmd

F32 = mybir.dt.float32
BF16 = mybir.dt.bfloat16

N_CORES = 8
P = 128
D = 1024          # d_model
S = D // P        # 8 subtiles of the contraction dim
H = 16            # heads
HD = 64           # head dim
B = 2
L = 2048
LQ = L * B // N_CORES   # 512 query rows per core
LK = L                  # key rows per core (full batch slice)
KTS = 128               # k tile
NKT = LK // KTS         # 16
SCALE = 1.0 / np.sqrt(HD)

import os
BENCH_LOOP = int(os.environ.get("BENCH_LOOP", "1"))


def _build(has_bias):
    nc = bacc.Bacc("TRN2", target_bir_lowering=False, debug=False, num_devices=1)

    def din(name, shape, dt):
        return nc.dram_tensor(name, shape, dt, kind="ExternalInput").ap()

    qT_d = din("qT", (P, S * LQ), BF16)
    kT_d = din("kT", (P, 4 * S * 512), BF16)
    vT_d = din("vT", (P, 4 * S * 512), BF16)
    wq_d = din("wq", (P, 2 * S * 512), BF16)
    wk_d = din("wk", (P, 2 * S * 512), BF16)
    wv_d = din("wv", (P, 2 * S * 512), BF16)
    wo_d = din("wo", (P, 2 * S * 512), BF16)
    bias_d = {}
    for nm in ("bq", "bk", "bv", "bo"):
        if has_bias[nm]:
            bias_d[nm] = din(nm, (1, D), F32)
    outT_d = nc.dram_tensor("outT", (P, S * LQ), F32, kind="ExternalOutput").ap()

    qT_ap = qT_d.rearrange("p (s q) -> p s q", s=S)
    kT_ap = kT_d.rearrange("p (c s k) -> p c s k", c=4, s=S)
    vT_ap = vT_d.rearrange("p (c s k) -> p c s k", c=4, s=S)
    wq_ap = wq_d.rearrange("p (h s o) -> p h s o", h=2, s=S)
    wk_ap = wk_d.rearrange("p (h s o) -> p h s o", h=2, s=S)
    wv_ap = wv_d.rearrange("p (h s o) -> p h s o", h=2, s=S)
    wo_ap = wo_d.rearrange("p (h s o) -> p h s o", h=2, s=S)
    outT_ap = outT_d.rearrange("p (j q) -> p j q", j=S)

    with tile.TileContext(nc) as tc, ExitStack() as ctx:
        if BENCH_LOOP > 1:
            ctx.enter_context(tc.For_i(0, BENCH_LOOP, 1))

        # ---- persistent data tiles ----
        qt_pool = ctx.enter_context(tc.tile_pool(name="QT", bufs=1))
        kt_pool = ctx.enter_context(tc.tile_pool(name="KT", bufs=1))
        v_pool = ctx.enter_context(tc.tile_pool(name="V", bufs=1))
        QT_sb = qt_pool.tile([P, S, LQ], BF16)
        KT_sb = kt_pool.tile([P, S, LK], BF16)
        V_sb = v_pool.tile([P, NKT, D], BF16)

        any_bias = any(has_bias.values())
        bias_t = {}
        ones_t = None
        if any_bias:
            cpool = ctx.enter_context(tc.tile_pool(name="const", bufs=1))
            ones_t = cpool.tile([1, 512], F32, tag="ones")
            nc.vector.memset(ones_t[:], 1.0)
            for nm, d_ap in bias_d.items():
                t = cpool.tile([1, D], F32, tag=f"bias_{nm}")
                nc.sync.dma_start(t[:], d_ap)
                bias_t[nm] = t

        def bias_mm(ps_t, bias_name, o0, n_sz, o_on_partitions):
            if o_on_partitions:
                nc.tensor.matmul(ps_t, lhsT=bias_t[bias_name][0:1, o0:o0 + P],
                                 rhs=ones_t[0:1, :n_sz], start=False, stop=True)
            else:
                nc.tensor.matmul(ps_t, lhsT=ones_t[0:1, 0:P],
                                 rhs=bias_t[bias_name][0:1, o0:o0 + n_sz],
                                 start=False, stop=True)

        # psum pools (8 banks total: 2 proj + 4 energy + 2 av)
        ppsum = ctx.enter_context(tc.tile_pool(name="ppsum", bufs=2, space="PSUM"))
        e_psum = ctx.enter_context(tc.tile_pool(name="epsum", bufs=2, space="PSUM"))
        av_psum = ctx.enter_context(tc.tile_pool(name="avpsum", bufs=1, space="PSUM"))

        # PE warm-up: the HAM clock-gate boots at reduced rate; ~5us of dead
        # matmuls during the initial input DMAs un-throttle it so the first
        # projection matmuls run at full clock.
        warm_pool = ctx.enter_context(tc.tile_pool(name="warm", bufs=1))
        wrm = warm_pool.tile([P, 512], BF16, tag="wrm")
        nc.vector.memset(wrm[:], 0.0)
        for wk_i in range(12):
            wps = e_psum.tile([P, 2, LQ], F32, tag="e")
            for hh in range(2):
                nc.tensor.matmul(
                    wps[:, hh, :],
                    lhsT=wrm[:, 0:P],
                    rhs=wrm[:, :],
                    start=True,
                    stop=True,
                )

        # ---------------- Q projection (scoped: SBUF reused later) ----
        with tc.tile_pool(name="qstream", bufs=1) as qspool, \
             tc.tile_pool(name="qwpool", bufs=2) as qwpool:
            qin = qspool.tile([P, S, LQ], BF16, tag="qin")
            nc.sync.dma_start(qin[:], qT_ap)
            wq_h = []
            for wh in range(2):
                t = qwpool.tile([P, S, 512], BF16, tag="wq")
                nc.sync.dma_start(t[:], wq_ap[:, wh])
                wq_h.append(t)
            for j in range(S):
                ps = ppsum.tile([P, 1, 512], F32, tag="pp")
                w_t = wq_h[j // 4]
                jl = j % 4
                for s in range(S):
                    nc.tensor.matmul(
                        ps[:, 0, :LQ],
                        lhsT=w_t[:, s, jl * P:(jl + 1) * P],
                        rhs=qin[:, s, :],
                        start=(s == 0),
                        stop=(s == S - 1 and not has_bias["bq"]),
                    )
                if has_bias["bq"]:
                    bias_mm(ps[:, 0, :LQ], "bq", j * P, LQ, True)
                nc.scalar.copy(QT_sb[:, j, :], ps[:, 0, :LQ])

        # ---- attention-era pools (allocated after Q scope frees) ----
        wk_pool = ctx.enter_context(tc.tile_pool(name="wk", bufs=2))
        kin_pool = ctx.enter_context(tc.tile_pool(name="kin", bufs=1))
        wv_pool = ctx.enter_context(tc.tile_pool(name="wv", bufs=2))
        vin_pool = ctx.enter_context(tc.tile_pool(name="vin", bufs=1))
        wo_pool = ctx.enter_context(tc.tile_pool(name="wo", bufs=1))
        attn_pool = ctx.enter_context(tc.tile_pool(name="attn", bufs=3))
        tree_pool = ctx.enter_context(tc.tile_pool(name="tree", bufs=1))
        den_pool = ctx.enter_context(tc.tile_pool(name="den", bufs=1))
        r_pool = ctx.enter_context(tc.tile_pool(name="r", bufs=2))
        rb_pool = ctx.enter_context(tc.tile_pool(name="rb", bufs=2))
        ctx_pool = ctx.enter_context(tc.tile_pool(name="ctx", bufs=1))
        osb_pool = ctx.enter_context(tc.tile_pool(name="osb", bufs=2))

        ctx_sb = ctx_pool.tile([P, S, LQ], BF16)

        wk_h = []
        for wh in range(2):
            t = wk_pool.tile([P, S, 512], BF16, tag="wk")
            nc.sync.dma_start(t[:], wk_ap[:, wh])
            wk_h.append(t)
        wv_h = []
        for wh in range(2):
            t = wv_pool.tile([P, S, 512], BF16, tag="wv")
            nc.sync.dma_start(t[:], wv_ap[:, wh])
            wv_h.append(t)

        kin_cur = [None]
        vin_cur = [None]

        def dma_kin(kn):
            t = kin_pool.tile([P, S, 512], BF16, tag="kin")
            nc.sync.dma_start(t[:], kT_ap[:, kn])
            kin_cur[0] = t

        def dma_vin(kn):
            t = vin_pool.tile([P, S, 512], BF16, tag="vin")
            nc.sync.dma_start(t[:], vT_ap[:, kn])
            vin_cur[0] = t

        def k_chunk_quarter(kn, jq):
            """Project K columns [kn*512,(kn+1)*512) for o-tiles 2jq,2jq+1."""
            kin = kin_cur[0]
            for j in (2 * jq, 2 * jq + 1):
                ps = ppsum.tile([P, 1, 512], F32, tag="pp")
                w_t = wk_h[j // 4]
                jl = j % 4
                for s in range(S):
                    nc.tensor.matmul(
                        ps[:, 0, :],
                        lhsT=w_t[:, s, jl * P:(jl + 1) * P],
                        rhs=kin[:, s, :],
                        start=(s == 0),
                        stop=(s == S - 1 and not has_bias["bk"]),
                    )
                if has_bias["bk"]:
                    bias_mm(ps[:, 0, :], "bk", j * P, 512, True)
                nc.scalar.copy(KT_sb[:, j, kn * 512:(kn + 1) * 512], ps[:, 0, :])

        def v_chunk_quarter(kn, kt4):
            """Project V rows for k-tile kn*4+kt4 (128 rows, all 1024 cols)."""
            vin = vin_cur[0]
            kt = kn * 4 + kt4
            for t in range(2):
                ps = ppsum.tile([P, 1, 512], F32, tag="pp")
                for s in range(S):
                    nc.tensor.matmul(
                        ps[:, 0, :],
                        lhsT=vin[:, s, kt4 * P:(kt4 + 1) * P],
                        rhs=wv_h[t][:, s, :],
                        start=(s == 0),
                        stop=(s == S - 1 and not has_bias["bv"]),
                    )
                if has_bias["bv"]:
                    bias_mm(ps[:, 0, :], "bv", t * 512, 512, False)
                nc.scalar.copy(V_sb[:, kt, t * 512:(t + 1) * 512], ps[:, 0, :])

        # ---------------- attention ----------------
        def softmax_kt(kt):
            """Energy (16 heads, row-packed pairs) -> exp -> normalized attn."""
            attn_t = attn_pool.tile([P, H, LQ], BF16, tag="attn")
            for g in range(8):
                eps = e_psum.tile([P, 2, LQ], F32, tag="e")
                for hh in range(2):
                    p0 = HD * hh
                    nc.tensor.matmul(
                        eps[:, hh, :],
                        lhsT=KT_sb[p0:p0 + HD, g, kt * KTS:(kt + 1) * KTS],
                        rhs=QT_sb[p0:p0 + HD, g, :],
                        start=True,
                        stop=True,
                    )
                nc.scalar.activation(attn_t[:, g * 2:(g + 1) * 2, :], eps[:],
                                     mybir.ActivationFunctionType.Exp,
                                     scale=float(SCALE))
            # den = sum over heads (bf16 tree at DVE 2x; final add f32)
            t1 = tree_pool.tile([P, 4, LQ], BF16)
            with nc.allow_low_precision(reason="bf16 head-sum tree"):
                nc.vector.tensor_add(t1[:], attn_t[:, 0:4, :], attn_t[:, 4:8, :])
                nc.vector.tensor_add(t1[:], t1[:], attn_t[:, 8:12, :])
                nc.vector.tensor_add(t1[:], t1[:], attn_t[:, 12:16, :])
                nc.vector.tensor_add(t1[:, 0:2, :], t1[:, 0:2, :], t1[:, 2:4, :])
            den = den_pool.tile([P, LQ], F32)
            nc.vector.tensor_add(den[:], t1[:, 0, :], t1[:, 1, :])
            r32 = r_pool.tile([P, LQ], F32, tag="r")
            nc.vector.reciprocal_approx_fast(r32[:], den[:])
            rb = rb_pool.tile([P, LQ], BF16, tag="rb")
            with nc.allow_low_precision(reason="bf16 reciprocal"):
                nc.vector.tensor_copy(rb[:], r32[:])
            nc.vector.tensor_mul(
                attn_t[:], attn_t[:],
                rb[:, None, :].to_broadcast((P, H, LQ)))
            return attn_t

        def av_group(u, c0, attn_list, first):
            """One avp tile: heads 4u..4u+3, full q, over 2 k-tiles."""
            avp = av_psum.tile([P, 2, LQ], F32, tag="av")
            for ci in range(2):
                kt = c0 + ci
                for hh in range(4):
                    h = 4 * u + hh
                    i, p0 = hh // 2, HD * (hh % 2)
                    nc.tensor.matmul(
                        avp[p0:p0 + HD, i, :],
                        lhsT=V_sb[:, kt, h * HD:(h + 1) * HD],
                        rhs=attn_list[ci][:, h, :],
                        start=(ci == 0),
                        stop=(ci == 1),
                    )
            with nc.allow_low_precision(reason="bf16 ctx accumulate"):
                if first:
                    nc.vector.tensor_copy(ctx_sb[:, 2 * u:2 * u + 2, :],
                                          avp[:, :, :])
                else:
                    nc.vector.tensor_add(ctx_sb[:, 2 * u:2 * u + 2, :],
                                         ctx_sb[:, 2 * u:2 * u + 2, :],
                                         avp[:, :, :])

        wo_tiles = []

        def dma_wo0():
            t = wo_pool.tile([P, S, 512], BF16, tag="wo")
            nc.sync.dma_start(t[:], wo_ap[:, 0])
            wo_tiles.append(t)

        # filler schedule: per-kt projection quarters + input DMAs
        def proj_filler(kt):
            if kt == 0:
                dma_kin(1)
                v_chunk_quarter(0, 0); v_chunk_quarter(0, 1)
            elif kt == 1:
                v_chunk_quarter(0, 2); v_chunk_quarter(0, 3)
                dma_vin(1)
            elif kt in (2, 3, 6, 7, 10, 11):
                kn = kt // 4 + 1
                jq0 = 0 if kt % 4 == 2 else 2
                k_chunk_quarter(kn, jq0); k_chunk_quarter(kn, jq0 + 1)
                if kt in (3, 7):
                    dma_kin(kn + 1)
            elif kt in (4, 5, 8, 9, 12, 13):
                kn = kt // 4
                kt40 = 0 if kt % 4 == 0 else 2
                v_chunk_quarter(kn, kt40); v_chunk_quarter(kn, kt40 + 1)
                if kt in (5, 9):
                    dma_vin(kn + 1)
                if kt == 12:
                    dma_wo0()
            # kt 14, 15: no proj work left

        # prologue: K chunk 0 (all 8 o-tiles)
        dma_kin(0)
        dma_vin(0)
        k_chunk_quarter(0, 0); k_chunk_quarter(0, 1)
        k_chunk_quarter(0, 2); k_chunk_quarter(0, 3)

        prev = None  # (c0, [attn_kt0, attn_kt1])
        for p in range(8):
            c0 = 2 * p
            cur = []
            for ci in range(2):
                kt = c0 + ci
                cur.append(softmax_kt(kt))
                if prev is not None:
                    for u in (2 * ci, 2 * ci + 1):
                        av_group(u, prev[0], prev[1], prev[0] == 0)
                proj_filler(kt)
            prev = (c0, cur)
        # PE warm-keeper: ~4us of dead matmuls run while the DVE finishes
        # the last normalize, so the HAM clock-gate stays at full rate for
        # the final AV groups + output projection.
        for wk_i in range(10):
            wps = e_psum.tile([P, 2, LQ], F32, tag="e")
            for hh in range(2):
                p0 = HD * hh
                nc.tensor.matmul(
                    wps[:, hh, :],
                    lhsT=KT_sb[p0:p0 + HD, wk_i % S, 0:KTS],
                    rhs=QT_sb[p0:p0 + HD, wk_i % S, :],
                    start=True,
                    stop=True,
                )
        for u in range(4):
            av_group(u, prev[0], prev[1], False)

        # ---------------- output projection ----------------
        for j4 in range(2):
            if j4 == 0 and wo_tiles:
                woh = wo_tiles[0]
            else:
                woh = wo_pool.tile([P, S, 512], BF16, tag="wo")
                nc.sync.dma_start(woh[:], wo_ap[:, j4])
            for j2 in range(2):
                po = e_psum.tile([P, 2, LQ], F32, tag="e")
                for jj in range(2):
                    j = j4 * 4 + j2 * 2 + jj
                    jl = j2 * 2 + jj
                    for s in range(S):
                        nc.tensor.matmul(
                            po[:, jj, :],
                            lhsT=woh[:, s, jl * P:(jl + 1) * P],
                            rhs=ctx_sb[:, s, :],
                            start=(s == 0),
                            stop=(s == S - 1 and not has_bias["bo"]),
                        )
                    if has_bias["bo"]:
                        bias_mm(po[:, jj, :], "bo", j * P, LQ, True)
                osb = osb_pool.tile([P, 2, LQ], F32, tag="osb")
                nc.scalar.copy(osb[:], po[:])
                j0 = j4 * 4 + j2 * 2
                nc.sync.dma_start(outT_ap[:, j0:j0 + 2, :], osb[:])

    nc.compile()
    return nc


_cache = {}


def _get_program(has_bias):
    key = (BENCH_LOOP, tuple(sorted(has_bias.items())))
    if key not in _cache:
        _cache[key] = _build(has_bias)
    return _cache[key]


def _part_major(x):
    n = x.shape[1]
    return np.ascontiguousarray(
        x.reshape(S, P, n).transpose(1, 0, 2).reshape(P, S * n))


def _chunked(x, width=512):
    """[D, N] -> [P, N//width, S, width] per-chunk contiguous layout."""
    n = x.shape[1]
    nch = n // width
    y = x.reshape(S, P, nch, width).transpose(1, 2, 0, 3)
    return np.ascontiguousarray(y.reshape(P, nch * S * width))


def _bf16(x):
    import ml_dtypes
    return np.ascontiguousarray(x).astype(ml_dtypes.bfloat16)


def prepare_inputs(query, key, value, Wq_w, Wq_b, Wk_w, Wk_b, Wv_w, Wv_b,
                   Wo_w, Wo_b):
    query = np.asarray(query, dtype=np.float32)
    key = np.asarray(key, dtype=np.float32)
    value = np.asarray(value, dtype=np.float32)
    w = {
        "wq": _bf16(_chunked(np.ascontiguousarray(np.asarray(Wq_w, np.float32).T))),
        "wk": _bf16(_chunked(np.ascontiguousarray(np.asarray(Wk_w, np.float32).T))),
        "wv": _bf16(_chunked(np.ascontiguousarray(np.asarray(Wv_w, np.float32).T))),
        "wo": _bf16(_chunked(np.ascontiguousarray(np.asarray(Wo_w, np.float32).T))),
    }
    biases = {"bq": np.asarray(Wq_b, np.float32), "bk": np.asarray(Wk_b, np.float32),
              "bv": np.asarray(Wv_b, np.float32), "bo": np.asarray(Wo_b, np.float32)}
    has_bias = {nm: bool(np.any(b)) for nm, b in biases.items()}

    kT = [_bf16(_chunked(np.ascontiguousarray(key[b].T))) for b in range(B)]
    vT = [_bf16(_chunked(np.ascontiguousarray(value[b].T))) for b in range(B)]

    in_maps = []
    for c in range(N_CORES):
        b, qc = c // (N_CORES // B), c % (N_CORES // B)
        qslice = query[b, qc * LQ:(qc + 1) * LQ, :]
        m = {
            "qT": _bf16(_part_major(np.ascontiguousarray(qslice.T))),
            "kT": kT[b],
            "vT": vT[b],
            **w,
        }
        for nm, hb in has_bias.items():
            if hb:
                m[nm] = biases[nm].reshape(1, D)
        in_maps.append(m)
    return in_maps, has_bias


def gather_output(results):
    out = np.empty((B, L, D), dtype=np.float32)
    for c in range(N_CORES):
        b, qc = c // (N_CORES // B), c % (N_CORES // B)
        oT = results[c]["outT"].reshape(P, S, LQ).transpose(1, 0, 2).reshape(D, LQ)
        out[b, qc * LQ:(qc + 1) * LQ, :] = oT.T
    return out


def kernel(**inputs) -> np.ndarray:
    in_maps, has_bias = prepare_inputs(**inputs)
    nc = _get_program(has_bias)
    res = run_bass_kernel_spmd(nc, in_maps, list(range(N_CORES)))
    return gather_output(res.results)
